# revision 6
# baseline (speedup 1.0000x reference)
"""Trainium2 Bass kernel: DepthSeparableConv2d (dw3x3 + BN + ReLU + map-cut,
pw 1x1 + BN + ReLU + map-cut), data-parallel over batch on 8 NeuronCores.

Host side folds all the small weight algebra (BN scales into conv weights,
pw transpose, biases) in numpy; the device kernel is a pure streaming
pipeline per core (4 images):

  - depthwise 3x3 conv as 9 diagonal-matmul "taps" on the TensorEngine
    (bf16 diag weights pre-scaled by the BN1 scale, bf16 activations, fp32
    PSUM accumulation); zero padding realized by AP sub-ranges + strided
    PSUM outputs, so the input DMA stays fully contiguous.
  - PSUM tiles are 2-bank [128,1024] pairs; dw pairs drain in ONE
    Scalar/Vector instruction (relu(psum+bias), bf16), pw pairs drain
    per 448-px half (lower latency for PSUM recycling).
  - dw map-cut: per-pair reduce_max on VectorE; keep = (max >= 4.0) folded
    into the pointwise lhsT (rows scaled by keep).
  - pointwise 1x1: 2 chunks of 128 out-channels, bf16 matmuls, drain
    relu(psum+bias2) -> bf16 z, streamed straight out to HBM per pair.
  - pw map-cut is applied ON HOST (z fully available there): exact
    max(map) >= 0.001 test in numpy.  Output DMA is bf16 (half traffic);
    host casts to fp32.
  - schedule: dw0 pw0 dw1 pw1 dw2 pw2m0 dw3 pw2m1 pw3m0 pw3m1 -- pw2's
    m=1 chunk fills the PE gap while image 3's drain->keep->lhsT-scale
    chain completes, and pw3's output DMA starts as early as possible.
"""

import numpy as np

B, C_IN, C_OUT, H, W = 32, 128, 256, 56, 56
N_CORES = 8
BPC = B // N_CORES          # images per core
HW = H * W                  # 3136
TILE_ROWS = 8               # output rows per 448-px sub-tile
NT = H // TILE_ROWS         # 7 sub-tiles per image
TN = TILE_ROWS * W          # 448 pixels per sub-tile
BN_EPS = 1e-5
DW_THRESH = 4.0
PW_THRESH = 0.001

# pairs of 448-px sub-tiles sharing one 2-bank PSUM tile
PAIRS = [(0, 1), (2, 3), (4, 5), (6,)]

# tap order: (0,0) first so the start=True matmul covers the full tile
TAPS = [(0, 0), (-1, 0), (1, 0), (0, -1), (0, 1),
        (-1, -1), (-1, 1), (1, -1), (1, 1)]

_CACHE = {}


def _build():
    import concourse.bacc as bacc
    import concourse.tile as tile
    import concourse.mybir as mybir

    f32 = mybir.dt.float32
    bf16 = mybir.dt.bfloat16
    Alu = mybir.AluOpType
    Act = mybir.ActivationFunctionType

    nc = bacc.Bacc("TRN2", target_bir_lowering=False, debug=False,
                   enable_asserts=True, num_devices=N_CORES)

    x_d = nc.dram_tensor("x", [BPC, C_IN, H, W], bf16, kind="ExternalInput").ap()
    dg_d = nc.dram_tensor("diags", [C_IN, 9, C_IN], bf16, kind="ExternalInput").ap()
    b1_d = nc.dram_tensor("bias1", [C_IN], f32, kind="ExternalInput").ap()
    lw_d = nc.dram_tensor("lhsTb", [C_IN, C_OUT], bf16, kind="ExternalInput").ap()
    b2_d = nc.dram_tensor("bias2", [C_OUT], f32, kind="ExternalInput").ap()
    z_d = nc.dram_tensor("z", [BPC, C_OUT, H, W], bf16, kind="ExternalOutput").ap()

    def vec(ap1d):
        return ap1d.rearrange("(c one) -> c one", one=1)

    with tile.TileContext(nc) as tc:
        with tc.tile_pool(name="const", bufs=1) as cp, \
             tc.tile_pool(name="xb", bufs=4) as xbp, \
             tc.tile_pool(name="y", bufs=4) as yp, \
             tc.tile_pool(name="z", bufs=6) as zp, \
             tc.tile_pool(name="small", bufs=8) as sp, \
             tc.tile_pool(name="dwps", bufs=2, space="PSUM") as dwps_pool, \
             tc.tile_pool(name="pwps", bufs=2, space="PSUM") as pwps_pool:

            # ---- startup DMAs: first-needed chunks first, spread queues ----
            dgt = cp.tile([128, 9 * 128], bf16)
            xb0 = xbp.tile([128, H, W], bf16, name="xbt")
            # tap-0 weights + x rows 0:18 gate the very first matmul pair
            nc.scalar.dma_start(dgt[:, 0:128], dg_d[:, 0, :])
            nc.sync.dma_start(xb0[:, 0:18, :], x_d[0][:, 0:18, :])
            nc.scalar.dma_start(dgt[:, 128:5 * 128]
                                .rearrange("c (t o) -> c t o", t=4),
                                dg_d[:, 1:5, :])
            nc.gpsimd.dma_start(dgt[:, 5 * 128:9 * 128]
                                .rearrange("c (t o) -> c t o", t=4),
                                dg_d[:, 5:9, :])
            nc.sync.dma_start(xb0[:, 18:40, :], x_d[0][:, 18:40, :])
            nc.sync.dma_start(xb0[:, 40:H, :], x_d[0][:, 40:H, :])

            # warm the PE HAM clock while the first DMAs are in flight
            warm = cp.tile([128, 448], bf16)
            nc.vector.memset(warm[:], 0.0)
            wps = pwps_pool.tile([128, 1024], f32, name="pwps")
            for _ in range(6):
                nc.tensor.matmul(wps[:, 0:448], warm[:, 0:128], warm[:],
                                 start=True, stop=True)

            bias1 = cp.tile([128, 1], f32)
            nc.gpsimd.dma_start(bias1[:], vec(b1_d))
            lhsT_base = cp.tile([128, C_OUT], bf16)
            nc.gpsimd.dma_start(lhsT_base[:], lw_d)
            bias2 = []
            for m in range(2):
                bb = cp.tile([128, 1], f32, name=f"bias2_{m}")
                nc.gpsimd.dma_start(bb[:], vec(b2_d[m * 128:(m + 1) * 128]))
                bias2.append(bb)

            # alternating engine pickers for drains and output DMAs
            state = {"dr": 0, "dma": 0}

            def drain_engine():
                state["dr"] += 1
                return nc.scalar if state["dr"] % 2 else nc.vector

            def dma_engine():
                state["dma"] += 1
                return nc.sync if state["dma"] % 2 else nc.gpsimd

            def emit_dw(n, xb):
                """depthwise conv + drain + keep1 chain; returns (yb, lhsTm)."""
                yb = yp.tile([128, HW], bf16, name="ybt")
                partdw = sp.tile([128, len(PAIRS)], f32, name="partdw")
                for pi, pair in enumerate(PAIRS):
                    ps = dwps_pool.tile([128, 1024], f32, name="dwps")
                    for t_idx, (di, dj) in enumerate(TAPS):
                        for k, tt in enumerate(pair):
                            r0 = tt * TILE_ROWS
                            rlo = max(0, r0 + di)
                            rhi = min(H, r0 + TILE_ROWS + di)
                            clo, chi = max(0, dj), min(W, W + dj)
                            rhs = xb[:, rlo:rhi, clo:chi]
                            ps3 = ps[:, k * 512:k * 512 + TN].rearrange(
                                "c (h w) -> c h w", h=TILE_ROWS)
                            out = ps3[:, rlo - di - r0:rhi - di - r0,
                                      clo - dj:chi - dj]
                            nc.tensor.matmul(
                                out, dgt[:, t_idx * 128:(t_idx + 1) * 128], rhs,
                                start=(t_idx == 0), stop=(t_idx == 8))
                    npair = len(pair)
                    c0 = pair[0] * TN
                    c1 = c0 + npair * TN
                    src = ps[:, 0:npair * 512].rearrange(
                        "c (b x) -> c b x", b=npair)[:, :, 0:TN]
                    dst = yb[:, c0:c1].rearrange("c (b x) -> c b x", b=npair)
                    # the last pair's drain is on the keep-chain: keep it on
                    # Vector so the whole chain runs back-to-back on one queue
                    eng = nc.vector if npair == 1 else drain_engine()
                    if eng is nc.scalar:
                        nc.scalar.activation(dst, src, Act.Relu,
                                             bias=bias1[:], scale=1.0)
                    else:
                        nc.vector.tensor_scalar(dst, src, bias1[:], 0.0,
                                                Alu.add, Alu.max)
                    nc.vector.tensor_reduce(partdw[:, pi:pi + 1], yb[:, c0:c1],
                                            axis=mybir.AxisListType.X,
                                            op=Alu.max)
                mx1 = sp.tile([128, 1], f32, name="mx1")
                nc.vector.tensor_reduce(mx1[:], partdw[:],
                                        axis=mybir.AxisListType.X, op=Alu.max)
                keep1 = sp.tile([128, 1], f32, name="keep1")
                nc.vector.tensor_scalar(keep1[:], mx1[:], float(DW_THRESH),
                                        None, Alu.is_ge)
                lhsTm = sp.tile([128, C_OUT], bf16, name="lhsTm")
                nc.vector.tensor_scalar(lhsTm[:], lhsT_base[:], keep1[:],
                                        None, Alu.mult)
                return yb, lhsTm

            def emit_pw_chunk(n, m, yb, lhsTm, pools):
                """one 128-out-channel chunk: matmuls + per-tile drains +
                per-pair output DMA.  pools cycles PSUM pools per pair."""
                zrow = z_d[n, m * 128:(m + 1) * 128].rearrange(
                    "c h w -> c (h w)")
                for pj, pair in enumerate(PAIRS):
                    pool = pools[pj % len(pools)]
                    ps = pool.tile([128, 1024], f32,
                                   name="dwps" if pool is dwps_pool
                                   else "pwps")
                    npair = len(pair)
                    zt = zp.tile([128, npair * TN], bf16, name="zt")
                    for k, tt in enumerate(pair):
                        nc.tensor.matmul(
                            ps[:, k * 512:k * 512 + TN],
                            lhsTm[:, m * 128:(m + 1) * 128],
                            yb[:, tt * TN:(tt + 1) * TN],
                            start=True, stop=True)
                    for k, tt in enumerate(pair):
                        src = ps[:, k * 512:k * 512 + TN]
                        dst = zt[:, k * TN:(k + 1) * TN]
                        eng = drain_engine()
                        if eng is nc.scalar:
                            nc.scalar.activation(dst, src, Act.Relu,
                                                 bias=bias2[m][:], scale=1.0)
                        else:
                            nc.vector.tensor_scalar(dst, src, bias2[m][:], 0.0,
                                                    Alu.add, Alu.max)
                    c0 = pair[0] * TN
                    dma_engine().dma_start(zrow[:, c0:c0 + npair * TN], zt[:])

            lhsTms = [None] * BPC
            ybs = [None] * BPC

            for n in range(BPC):
                if n == 0:
                    xb = xb0
                else:
                    xb = xbp.tile([128, H, W], bf16, name="xbt")
                    nc.sync.dma_start(xb[:, 0:28, :], x_d[n][:, 0:28, :])
                    nc.sync.dma_start(xb[:, 28:H, :], x_d[n][:, 28:H, :])
                ybs[n], lhsTms[n] = emit_dw(n, xb)
                if n < 2:
                    emit_pw_chunk(n, 0, ybs[n], lhsTms[n], [pwps_pool])
                    emit_pw_chunk(n, 1, ybs[n], lhsTms[n], [pwps_pool])
                elif n == 2:
                    emit_pw_chunk(2, 0, ybs[2], lhsTms[2], [pwps_pool])
            # image 2's m=1 chunk fills the PE gap while image 3's
            # drain->keep->lhsT chain completes; then image 3's pw with
            # both PSUM pools for deeper pipelining
            emit_pw_chunk(2, 1, ybs[2], lhsTms[2], [pwps_pool])
            emit_pw_chunk(3, 0, ybs[3], lhsTms[3], [pwps_pool, dwps_pool])
            emit_pw_chunk(3, 1, ybs[3], lhsTms[3], [pwps_pool, dwps_pool])

    nc.compile()
    return nc


def _get_nc():
    if "nc" not in _CACHE:
        _CACHE["nc"] = _build()
    return _CACHE["nc"]


def _fold_weights(inputs):
    """Host-side numpy prep of all the small weight algebra."""
    dw_w = np.asarray(inputs["dw_w"], np.float64).reshape(C_IN, 9)
    dw_b = np.asarray(inputs["dw_b"], np.float64)
    g1 = np.asarray(inputs["bn1_g"], np.float64)
    b1 = np.asarray(inputs["bn1_b"], np.float64)
    m1 = np.asarray(inputs["bn1_m"], np.float64)
    v1 = np.asarray(inputs["bn1_v"], np.float64)
    pw_w = np.asarray(inputs["pw_w"], np.float64)
    pw_b = np.asarray(inputs["pw_b"], np.float64)
    g2 = np.asarray(inputs["bn2_g"], np.float64)
    b2 = np.asarray(inputs["bn2_b"], np.float64)
    m2 = np.asarray(inputs["bn2_m"], np.float64)
    v2 = np.asarray(inputs["bn2_v"], np.float64)

    s1 = g1 / np.sqrt(v1 + BN_EPS)
    bias1 = (s1 * (dw_b - m1) + b1).astype(np.float32)
    dws = dw_w * s1[:, None]                      # [C_IN, 9]
    diags = np.zeros((C_IN, 9, C_IN), np.float32)
    idx = np.arange(C_IN)
    for t, (di, dj) in enumerate(TAPS):
        k = (di + 1) * 3 + (dj + 1)
        diags[idx, t, idx] = dws[:, k]

    s2 = g2 / np.sqrt(v2 + BN_EPS)
    bias2 = (s2 * (pw_b - m2) + b2).astype(np.float32)
    lhsTb = (pw_w * s2[:, None]).T.astype(np.float32)   # [C_IN, C_OUT]

    import ml_dtypes
    return {
        "diags": np.ascontiguousarray(diags.astype(ml_dtypes.bfloat16)),
        "bias1": bias1,
        "lhsTb": np.ascontiguousarray(lhsTb.astype(ml_dtypes.bfloat16)),
        "bias2": bias2,
    }


def _make_in_maps(inputs):
    import ml_dtypes
    x = np.asarray(inputs["x"]).astype(ml_dtypes.bfloat16)
    folded = _fold_weights(inputs)
    in_maps = []
    for c in range(N_CORES):
        m = {"x": np.ascontiguousarray(x[c * BPC:(c + 1) * BPC])}
        m.update(folded)
        in_maps.append(m)
    return in_maps


def kernel(**inputs):
    from concourse.bass_utils import run_bass_kernel_spmd

    nc = _get_nc()
    in_maps = _make_in_maps(inputs)
    res = run_bass_kernel_spmd(nc, in_maps, core_ids=list(range(N_CORES)))
    _CACHE["last_results"] = res
    z = np.concatenate([np.asarray(res.results[c]["z"])
                        for c in range(N_CORES)], axis=0).astype(np.float32)
    # pw map-cut on host: zero any (n, o) map whose max is below PW_THRESH
    mx = z.max(axis=(2, 3))
    z *= (mx >= PW_THRESH).astype(np.float32)[:, :, None, None]
    return z


# revision 12
# speedup vs baseline: 1.0175x; 1.0175x over previous
"""Trainium2 Bass kernel: DepthSeparableConv2d (dw3x3 + BN + ReLU + map-cut,
pw 1x1 + BN + ReLU + map-cut), data-parallel over batch on 8 NeuronCores.

Host side folds all the small weight algebra (BN scales into conv weights,
pw transpose, biases) in numpy; the device kernel is a pure streaming
pipeline per core (4 images):

  - depthwise 3x3 conv as 9 diagonal-matmul "taps" on the TensorEngine
    (bf16 diag weights pre-scaled by the BN1 scale, bf16 activations, fp32
    PSUM accumulation); zero padding realized by AP sub-ranges + strided
    PSUM outputs, so the input DMA stays fully contiguous.
  - PSUM tiles are 2-bank [128,1024] pairs; dw pairs drain in ONE
    Scalar/Vector instruction (relu(psum+bias), bf16), pw pairs drain
    per 448-px half (lower latency for PSUM recycling).
  - dw map-cut: per-pair reduce_max on VectorE with an incremental partial
    max so the per-image keep chain after the last drain is 4 tiny ops;
    keep = (max >= 4.0) folded into the pointwise lhsT (two [128,128]
    halves so each pw chunk gates on its own half).
  - pointwise 1x1: 2 chunks of 128 out-channels, bf16 matmuls, drain
    relu(psum+bias2) -> bf16 z, streamed straight out to HBM per pair.
  - pw map-cut is applied ON HOST (z fully available there): exact
    max(map) >= 0.001 test in numpy.  Output DMA is bf16 (half traffic);
    host casts to fp32.
  - schedule: dw0 pw0 dw1 pw1 dw2 pw2m0 dw3 pw2m1 pw3m0 pw3m1 -- pw2's
    m=1 chunk fills the PE bubble while image 3's keep chain completes.
    The endgame (pw2m1+pw3) rotates drains over Vector/Scalar/GpSimd and
    PSUM pairs over both pools for deeper pipelining.
"""

import numpy as np

B, C_IN, C_OUT, H, W = 32, 128, 256, 56, 56
N_CORES = 8
BPC = B // N_CORES          # images per core
HW = H * W                  # 3136
TILE_ROWS = 8               # output rows per 448-px sub-tile
NT = H // TILE_ROWS         # 7 sub-tiles per image
TN = TILE_ROWS * W          # 448 pixels per sub-tile
BN_EPS = 1e-5
DW_THRESH = 4.0
PW_THRESH = 0.001

# pairs of 448-px sub-tiles sharing one 2-bank PSUM tile
PAIRS = [(0, 1), (2, 3), (4, 5), (6,)]

# tap order: (0,0) first so the start=True matmul covers the full tile
TAPS = [(0, 0), (-1, 0), (1, 0), (0, -1), (0, 1),
        (-1, -1), (-1, 1), (1, -1), (1, 1)]

_CACHE = {}


def _build():
    import concourse.bacc as bacc
    import concourse.tile as tile
    import concourse.mybir as mybir

    f32 = mybir.dt.float32
    bf16 = mybir.dt.bfloat16
    Alu = mybir.AluOpType
    Act = mybir.ActivationFunctionType

    nc = bacc.Bacc("TRN2", target_bir_lowering=False, debug=False,
                   enable_asserts=True, num_devices=N_CORES)

    x_d = nc.dram_tensor("x", [BPC, C_IN, H, W], bf16, kind="ExternalInput").ap()
    dg_d = nc.dram_tensor("diags", [C_IN, 9, C_IN], bf16, kind="ExternalInput").ap()
    b1_d = nc.dram_tensor("bias1", [C_IN], f32, kind="ExternalInput").ap()
    lw_d = nc.dram_tensor("lhsTb", [C_IN, C_OUT], bf16, kind="ExternalInput").ap()
    b2_d = nc.dram_tensor("bias2", [C_OUT], f32, kind="ExternalInput").ap()
    z_d = nc.dram_tensor("z", [BPC, C_OUT, H, W], bf16, kind="ExternalOutput").ap()

    def vec(ap1d):
        return ap1d.rearrange("(c one) -> c one", one=1)

    with tile.TileContext(nc) as tc:
        with tc.tile_pool(name="const", bufs=1) as cp, \
             tc.tile_pool(name="xb", bufs=4) as xbp, \
             tc.tile_pool(name="y", bufs=4) as yp, \
             tc.tile_pool(name="z", bufs=6) as zp, \
             tc.tile_pool(name="small", bufs=10) as sp, \
             tc.tile_pool(name="dwps", bufs=2, space="PSUM") as dwps_pool, \
             tc.tile_pool(name="pwps", bufs=2, space="PSUM") as pwps_pool:

            # ---- startup DMAs: first-needed chunks first, flat 2D APs so
            # descriptors stay large (per-partition-contiguous lines) ----
            dgt = cp.tile([128, 9 * 128], bf16)
            xb0 = xbp.tile([128, H, W], bf16, name="xbt")
            # tap-0 weights + x rows 0:18 gate the very first matmul pair
            nc.scalar.dma_start(dgt[:, 0:128], dg_d[:, 0, :])
            nc.sync.dma_start(xb0[:, 0:18, :], x_d[0][:, 0:18, :])
            # taps 1-8: contiguous 2048B per partition on both sides
            nc.scalar.dma_start(dgt[:, 128:9 * 128],
                                dg_d[:, 1:9, :].rearrange("c t o -> c (t o)"))
            nc.sync.dma_start(xb0[:, 18:40, :], x_d[0][:, 18:40, :])
            nc.sync.dma_start(xb0[:, 40:H, :], x_d[0][:, 40:H, :])

            # warm the PE HAM clock while the first DMAs are in flight
            warm = cp.tile([128, 448], bf16)
            nc.vector.memset(warm[:], 0.0)
            wps = pwps_pool.tile([128, 1024], f32, name="pwps")
            for _ in range(6):
                nc.tensor.matmul(wps[:, 0:448], warm[:, 0:128], warm[:],
                                 start=True, stop=True)

            bias1 = cp.tile([128, 1], f32)
            nc.gpsimd.dma_start(bias1[:], vec(b1_d))
            lhsT_base = cp.tile([128, C_OUT], bf16)
            nc.gpsimd.dma_start(lhsT_base[:], lw_d)
            bias2 = []
            for m in range(2):
                bb = cp.tile([128, 1], f32, name=f"bias2_{m}")
                nc.gpsimd.dma_start(bb[:], vec(b2_d[m * 128:(m + 1) * 128]))
                bias2.append(bb)

            # rotating engine pickers for drains and output DMAs
            state = {"dr": 0, "dma": 0}

            def drain_op(dst, src, bias, rot):
                state["dr"] += 1
                eng = rot[state["dr"] % len(rot)]
                if eng is nc.scalar:
                    nc.scalar.activation(dst, src, Act.Relu,
                                         bias=bias[:], scale=1.0)
                else:
                    eng.tensor_scalar(dst, src, bias[:], 0.0,
                                      Alu.add, Alu.max)

            def dma_engine():
                state["dma"] += 1
                return nc.sync if state["dma"] % 2 else nc.gpsimd

            def emit_dw(n, xb, pair_rot):
                """depthwise conv + drain + keep1 chain.
                Returns (yb, [lhsTm_half0, lhsTm_half1])."""
                yb = yp.tile([128, HW], bf16, name="ybt")
                partdw = sp.tile([128, len(PAIRS)], f32, name="partdw")
                mxa = sp.tile([128, 1], f32, name="mxa")
                for pi, pair in enumerate(PAIRS):
                    ps = dwps_pool.tile([128, 1024], f32, name="dwps")
                    for t_idx, (di, dj) in enumerate(TAPS):
                        for k, tt in enumerate(pair):
                            r0 = tt * TILE_ROWS
                            rlo = max(0, r0 + di)
                            rhi = min(H, r0 + TILE_ROWS + di)
                            clo, chi = max(0, dj), min(W, W + dj)
                            rhs = xb[:, rlo:rhi, clo:chi]
                            ps3 = ps[:, k * 512:k * 512 + TN].rearrange(
                                "c (h w) -> c h w", h=TILE_ROWS)
                            out = ps3[:, rlo - di - r0:rhi - di - r0,
                                      clo - dj:chi - dj]
                            nc.tensor.matmul(
                                out, dgt[:, t_idx * 128:(t_idx + 1) * 128], rhs,
                                start=(t_idx == 0), stop=(t_idx == 8))
                    npair = len(pair)
                    c0 = pair[0] * TN
                    c1 = c0 + npair * TN
                    src = ps[:, 0:npair * 512].rearrange(
                        "c (b x) -> c b x", b=npair)[:, :, 0:TN]
                    dst = yb[:, c0:c1].rearrange("c (b x) -> c b x", b=npair)
                    # the last (single-tile) pair is on the keep chain:
                    # keep it on Vector so the chain runs on one queue
                    if npair == 1:
                        nc.vector.tensor_scalar(dst, src, bias1[:], 0.0,
                                                Alu.add, Alu.max)
                    else:
                        drain_op(dst, src, bias1, pair_rot)
                    nc.vector.tensor_reduce(partdw[:, pi:pi + 1], yb[:, c0:c1],
                                            axis=mybir.AxisListType.X,
                                            op=Alu.max)
                    if pi == 2:
                        # partial max of pairs 0-2, off the critical chain
                        nc.vector.tensor_reduce(mxa[:], partdw[:, 0:3],
                                                axis=mybir.AxisListType.X,
                                                op=Alu.max)
                mx1 = sp.tile([128, 1], f32, name="mx1")
                nc.vector.tensor_max(mx1[:], mxa[:], partdw[:, 3:4])
                keep1 = sp.tile([128, 1], f32, name="keep1")
                nc.vector.tensor_scalar(keep1[:], mx1[:], float(DW_THRESH),
                                        None, Alu.is_ge)
                lhsTms = []
                for m in range(2):
                    lm = sp.tile([128, 128], bf16, name=f"lhsTm{m}")
                    nc.vector.tensor_scalar(
                        lm[:], lhsT_base[:, m * 128:(m + 1) * 128], keep1[:],
                        None, Alu.mult)
                    lhsTms.append(lm)
                return yb, lhsTms

            def emit_pw_chunk(n, m, yb, lhsTm, pools, rot):
                """one 128-out-channel chunk: matmuls + per-tile drains +
                per-pair output DMA.  pools cycles PSUM pools per pair."""
                zrow = z_d[n, m * 128:(m + 1) * 128].rearrange(
                    "c h w -> c (h w)")
                for pj, pair in enumerate(PAIRS):
                    pool = pools[pj % len(pools)]
                    ps = pool.tile([128, 1024], f32,
                                   name="dwps" if pool is dwps_pool
                                   else "pwps")
                    npair = len(pair)
                    zt = zp.tile([128, npair * TN], bf16, name="zt")
                    for k, tt in enumerate(pair):
                        nc.tensor.matmul(
                            ps[:, k * 512:k * 512 + TN], lhsTm[:],
                            yb[:, tt * TN:(tt + 1) * TN],
                            start=True, stop=True)
                    for k, tt in enumerate(pair):
                        drain_op(zt[:, k * TN:(k + 1) * TN],
                                 ps[:, k * 512:k * 512 + TN], bias2[m], rot)
                    c0 = pair[0] * TN
                    dma_engine().dma_start(zrow[:, c0:c0 + npair * TN], zt[:])

            VS = (nc.vector, nc.scalar)
            S_ = (nc.scalar,)
            lhsTms = [None] * BPC
            ybs = [None] * BPC

            for n in range(BPC):
                if n == 0:
                    xb = xb0
                else:
                    xb = xbp.tile([128, H, W], bf16, name="xbt")
                    nc.sync.dma_start(xb[:, 0:28, :], x_d[n][:, 0:28, :])
                    nc.sync.dma_start(xb[:, 28:H, :], x_d[n][:, 28:H, :])
                # image 3: paired drains all on Scalar so Vector is free
                # for the reduce/keep chain feeding straight into pw3
                ybs[n], lhsTms[n] = emit_dw(n, xb, S_ if n == 3 else VS)
                if n < 2:
                    emit_pw_chunk(n, 0, ybs[n], lhsTms[n][0], [pwps_pool], VS)
                    emit_pw_chunk(n, 1, ybs[n], lhsTms[n][1], [pwps_pool], VS)
                elif n == 2:
                    emit_pw_chunk(2, 0, ybs[2], lhsTms[2][0], [pwps_pool], VS)
            # image 2's m=1 chunk fills the PE bubble while image 3's
            # keep chain completes; endgame uses both PSUM pools + 3-way
            # drain rotation so nothing gates the PE or the output DMA
            both = [pwps_pool, dwps_pool]
            emit_pw_chunk(2, 1, ybs[2], lhsTms[2][1], both, VS)
            emit_pw_chunk(3, 0, ybs[3], lhsTms[3][0], both, VS)
            emit_pw_chunk(3, 1, ybs[3], lhsTms[3][1], both, VS)

    nc.compile()
    return nc


def _get_nc():
    if "nc" not in _CACHE:
        _CACHE["nc"] = _build()
    return _CACHE["nc"]


def _fold_weights(inputs):
    """Host-side numpy prep of all the small weight algebra."""
    dw_w = np.asarray(inputs["dw_w"], np.float64).reshape(C_IN, 9)
    dw_b = np.asarray(inputs["dw_b"], np.float64)
    g1 = np.asarray(inputs["bn1_g"], np.float64)
    b1 = np.asarray(inputs["bn1_b"], np.float64)
    m1 = np.asarray(inputs["bn1_m"], np.float64)
    v1 = np.asarray(inputs["bn1_v"], np.float64)
    pw_w = np.asarray(inputs["pw_w"], np.float64)
    pw_b = np.asarray(inputs["pw_b"], np.float64)
    g2 = np.asarray(inputs["bn2_g"], np.float64)
    b2 = np.asarray(inputs["bn2_b"], np.float64)
    m2 = np.asarray(inputs["bn2_m"], np.float64)
    v2 = np.asarray(inputs["bn2_v"], np.float64)

    s1 = g1 / np.sqrt(v1 + BN_EPS)
    bias1 = (s1 * (dw_b - m1) + b1).astype(np.float32)
    dws = dw_w * s1[:, None]                      # [C_IN, 9]
    diags = np.zeros((C_IN, 9, C_IN), np.float32)
    idx = np.arange(C_IN)
    for t, (di, dj) in enumerate(TAPS):
        k = (di + 1) * 3 + (dj + 1)
        diags[idx, t, idx] = dws[:, k]

    s2 = g2 / np.sqrt(v2 + BN_EPS)
    bias2 = (s2 * (pw_b - m2) + b2).astype(np.float32)
    lhsTb = (pw_w * s2[:, None]).T.astype(np.float32)   # [C_IN, C_OUT]

    import ml_dtypes
    return {
        "diags": np.ascontiguousarray(diags.astype(ml_dtypes.bfloat16)),
        "bias1": bias1,
        "lhsTb": np.ascontiguousarray(lhsTb.astype(ml_dtypes.bfloat16)),
        "bias2": bias2,
    }


def _make_in_maps(inputs):
    import ml_dtypes
    x = np.asarray(inputs["x"]).astype(ml_dtypes.bfloat16)
    folded = _fold_weights(inputs)
    in_maps = []
    for c in range(N_CORES):
        m = {"x": np.ascontiguousarray(x[c * BPC:(c + 1) * BPC])}
        m.update(folded)
        in_maps.append(m)
    return in_maps


def kernel(**inputs):
    from concourse.bass_utils import run_bass_kernel_spmd

    nc = _get_nc()
    in_maps = _make_in_maps(inputs)
    res = run_bass_kernel_spmd(nc, in_maps, core_ids=list(range(N_CORES)))
    _CACHE["last_results"] = res
    z = np.concatenate([np.asarray(res.results[c]["z"])
                        for c in range(N_CORES)], axis=0).astype(np.float32)
    # pw map-cut on host: zero any (n, o) map whose max is below PW_THRESH
    mx = z.max(axis=(2, 3))
    z *= (mx >= PW_THRESH).astype(np.float32)[:, :, None, None]
    return z
